# revision 4
# baseline (speedup 1.0000x reference)
"""Trainium2 Bass kernel for nn_CPLinear (CP-decomposed QKV projection with RoPE).

Computes, for x:(2,4096,2048) and CP-factor weights:
    A_t = x @ W_A_t  (per-token head coefficients),  B_t = x @ W_B_t (shared bases)
    q = einsum('bshr,bsrd->bshd', A_q, rope(B_q)) / 12
    k = A_k * rope(B_k)   (rank-1)
    v = A_v * B_v         (rank-1)

Strategy (8 cores, data-parallel over the 8192 tokens, 1024 tokens/core):
  - All 6 projections fused into one [2048 x 2016] bf16 matmul, split into two
    column halves (992 + 1024) so PSUM stays fully double-buffered
    (3-deep [128,1024] rotation) with zero eviction stalls.
  - Phase 1 computes half-A (A_q', A_k, A_v, B_k, B_v, B_q r0-3) for all 8
    token tiles; the first two tiles run k-major, paced by k-chunked W/x DMAs
    so the PE starts ~10us in instead of waiting for the full weight upload.
  - Phase 2 computes half-B (B_q r4-11) and runs the rank-12 q contraction as
    block-diagonal matmuls (8 tokens per matmul, K=96), woven between
    projection tiles so the scatter round-trip hides under PE work.
  - Two projection tiles share one block-diagonal batch (super-tile, 32 groups)
    halving the scatter DMA count; A'/roped-B_q bounce through DRAM with
    per-diagonal reads (3-dim AP limit).
  - q is written in raw block-diagonal layout and untangled on the host.
"""

import sys

for _p in ("/opt/trn_rl_repo",):
    if _p not in sys.path:
        sys.path.insert(0, _p)

import numpy as np
import ml_dtypes

BF16 = ml_dtypes.bfloat16

SH = 1024          # tokens per core
H = 2048           # hidden
KT = H // 128      # 16 k-tiles
NT = SH // 128     # 8 token tiles per core
NSUP = NT // 2     # 4 super-tiles (2 proj tiles each) for the BD contraction
NOUT = 2016        # fused projection output width
NA = 992           # half-A columns: [B_q r0-3 | A'q | A_k | A_v | B_k | B_v]
NB = 1024          # half-B columns: [B_q r4-11]
NH, HD, RQ = 16, 128, 12

_CACHE = {}


def make_nc():
    import concourse.bacc as bacc
    from concourse import mybir

    dt = mybir.dt

    nc = bacc.Bacc(
        "TRN2",
        target_bir_lowering=False,
        debug=False,
        enable_asserts=False,
        num_devices=8,
    )

    x_d = nc.dram_tensor("x", (H, SH), dt.bfloat16, kind="ExternalInput")  # pre-transposed host-side
    wa_d = nc.dram_tensor("wa", (KT, 128, NA), dt.bfloat16, kind="ExternalInput")
    wb_d = nc.dram_tensor("wb", (KT, 128, NB), dt.bfloat16, kind="ExternalInput")
    cos_d = nc.dram_tensor("cosr", (SH, 64), dt.bfloat16, kind="ExternalInput")
    sin_d = nc.dram_tensor("sinr", (SH, 64), dt.bfloat16, kind="ExternalInput")
    q_d = nc.dram_tensor("q", (NT, 128, NH * HD), dt.bfloat16, kind="ExternalOutput")
    k_d = nc.dram_tensor("k", (SH, NH * HD), dt.bfloat16, kind="ExternalOutput")
    v_d = nc.dram_tensor("v", (SH, NH * HD), dt.bfloat16, kind="ExternalOutput")
    return nc, (x_d, wa_d, wb_d, cos_d, sin_d, q_d, k_d, v_d)


def build_body(nc, tc, tensors):
    from contextlib import ExitStack

    from concourse import mybir

    dt = mybir.dt
    x_d, wa_d, wb_d, cos_d, sin_d, q_d, k_d, v_d = tensors

    with ExitStack() as ctx:
        P = ctx.enter_context
        const_pool = P(tc.tile_pool(name="const", bufs=1))
        scr_pool = P(tc.tile_pool(name="scr", bufs=NSUP, space="DRAM"))
        scr_d = [
            scr_pool.tile([256, 1728], dt.bfloat16, tag="scr", name=f"scr{j}")
            for j in range(NSUP)
        ]
        wa_sb = const_pool.tile([128, KT * NA], dt.bfloat16, tag="wa_sb")
        wb_sb = const_pool.tile([128, KT * NB], dt.bfloat16, tag="wb_sb")
        xT = const_pool.tile([128, KT * SH], dt.bfloat16, tag="xT")
        cos_sb = const_pool.tile([128, NT * 64], dt.bfloat16, tag="cos_sb")
        sin_sb = const_pool.tile([128, NT * 64], dt.bfloat16, tag="sin_sb")
        # block-diagonal operand holders, one pair per 2 super-tiles (rotate 2)
        lhs_bufs = [
            const_pool.tile([128, 4096], dt.bfloat16, tag=f"lhs{i}", name=f"lhs{i}")
            for i in range(2)
        ]
        bdr_bufs = [
            const_pool.tile([128, 4096], dt.bfloat16, tag=f"bdr{i}", name=f"bdr{i}")
            for i in range(2)
        ]

        # ---- startup DMAs ----
        nc.gpsimd.dma_start(
            out=cos_sb[:].rearrange("p (t n) -> p t n", t=NT),
            in_=cos_d[:].rearrange("(t p) n -> p t n", p=128),
        )
        nc.gpsimd.dma_start(
            out=sin_sb[:].rearrange("p (t n) -> p t n", t=NT),
            in_=sin_d[:].rearrange("(t p) n -> p t n", p=128),
        )
        for tl in lhs_bufs:
            nc.gpsimd.memset(tl[:], 0.0)
        # k-chunked half-A weight + x loads: the first matmul only needs chunk 0
        for kk in range(KT):
            nc.scalar.dma_start(
                out=wa_sb[:, kk * NA : (kk + 1) * NA], in_=wa_d[kk]
            )
            nc.sync.dma_start(
                out=xT[:, kk * SH : (kk + 1) * SH],
                in_=x_d[kk * 128 : (kk + 1) * 128, :],
            )

        stage_pool = P(tc.tile_pool(name="stage", bufs=NT))
        ps_pool = P(tc.tile_pool(name="ps", bufs=3, space="PSUM"))
        psq_pool = P(tc.tile_pool(name="psq", bufs=2, space="PSUM"))
        small_pool = P(tc.tile_pool(name="small", bufs=3))
        tmpb_pool = P(tc.tile_pool(name="tmpb", bufs=2))
        rope_pool = P(tc.tile_pool(name="rope", bufs=3))
        bqr_pool = P(tc.tile_pool(name="bqr", bufs=3))
        out_pool = P(tc.tile_pool(name="outs", bufs=3))

        stages = {}

        def half_mms(p, ps, w_sb, w_base, w_n, k_lo, k_hi):
            t0 = p * 128
            for kk in range(k_lo, k_hi):
                lh = xT[:, kk * SH + t0 : kk * SH + t0 + 128]
                st = kk == 0
                sp = kk == KT - 1
                nc.tensor.matmul(
                    ps[:, 0:512],
                    lh,
                    w_sb[:, kk * w_n + w_base : kk * w_n + w_base + 512],
                    start=st,
                    stop=sp,
                )
                nc.tensor.matmul(
                    ps[:, 512 : w_n - w_base],
                    lh,
                    w_sb[:, kk * w_n + w_base + 512 : (kk + 1) * w_n],
                    start=st,
                    stop=sp,
                )

        def post1(p, ps):
            """half-A evictions + ropeK + k/v for proj tile p."""
            t0 = p * 128
            stage = stage_pool.tile([128, 704], dt.bfloat16, tag="stage",
                                    name=f"stage{p}")
            smalls = small_pool.tile([128, 288], dt.bfloat16, tag="smalls")
            bkr = small_pool.tile([128, 128], dt.bfloat16, tag="bkr")
            tka = small_pool.tile([128, 64], dt.bfloat16, tag="tka")
            tkb = small_pool.tile([128, 64], dt.bfloat16, tag="tkb")
            nc.scalar.copy(stage[:], ps[:, 0:704])
            nc.scalar.copy(smalls[:], ps[:, 704:992])
            stages[p] = stage

            # rope on B_k (rank-1)
            cos_k = cos_sb[:, p * 64 : (p + 1) * 64]
            sin_k = sin_sb[:, p * 64 : (p + 1) * 64]
            bkv = smalls[:, 32:160].rearrange("p (two d) -> p two d", two=2)
            bkrv = bkr[:].rearrange("p (two d) -> p two d", two=2)
            nc.vector.tensor_mul(tka[:], bkv[:, 0], cos_k)
            nc.vector.tensor_mul(tkb[:], bkv[:, 1], sin_k)
            nc.vector.tensor_sub(bkrv[:, 0], tka[:], tkb[:])
            nc.vector.tensor_mul(tka[:], bkv[:, 1], cos_k)
            nc.vector.tensor_mul(tkb[:], bkv[:, 0], sin_k)
            nc.vector.tensor_add(bkrv[:, 1], tka[:], tkb[:])

            # k, v rank-1 broadcasts + outputs
            ksb = out_pool.tile([128, 2048], dt.bfloat16, tag="ksb")
            vsb = out_pool.tile([128, 2048], dt.bfloat16, tag="vsb")
            nc.vector.tensor_mul(
                ksb[:].rearrange("p (h d) -> p h d", h=NH),
                bkr[:].unsqueeze(1).broadcast_to([128, NH, 128]),
                smalls[:, 0:16].unsqueeze(2).broadcast_to([128, NH, 128]),
            )
            nc.vector.tensor_mul(
                vsb[:].rearrange("p (h d) -> p h d", h=NH),
                smalls[:, 160:288].unsqueeze(1).broadcast_to([128, NH, 128]),
                smalls[:, 16:32].unsqueeze(2).broadcast_to([128, NH, 128]),
            )
            nc.sync.dma_start(out=k_d[t0 : t0 + 128, :], in_=ksb[:])
            nc.sync.dma_start(out=v_d[t0 : t0 + 128, :], in_=vsb[:])

            # stage A' into the super-tile scratch (read back by l_v later)
            scr = scr_d[p // 2]
            ph = p % 2
            nc.gpsimd.dma_start(
                out=scr[ph * 128 : (ph + 1) * 128, 1536:1728],
                in_=stage[:, 512:704],
            )

        def rope_block(dst, src, cos_t, sin_t, r, ta, tb):
            sv = src.rearrange("p (r two d) -> p r two d", r=r, two=2)
            dv = dst.rearrange("p (r two d) -> p r two d", r=r, two=2)
            tav = ta.rearrange("p (r d) -> p r d", r=r)
            tbv = tb.rearrange("p (r d) -> p r d", r=r)
            p_lo = sv[:, :, 0]
            p_hi = sv[:, :, 1]
            nc.vector.tensor_mul(tav, p_lo, cos_t)
            nc.vector.tensor_mul(tbv, p_hi, sin_t)
            nc.vector.tensor_sub(dv[:, :, 0], tav, tbv)
            nc.vector.tensor_mul(tav, p_hi, cos_t)
            nc.vector.tensor_mul(tbv, p_lo, sin_t)
            nc.vector.tensor_add(dv[:, :, 1], tav, tbv)

        bqrs = {}

        def post2(p, ps):
            """half-B eviction, rope on B_q, bounce write for proj tile p."""
            tmpb = tmpb_pool.tile([128, 1024], dt.bfloat16, tag="tmpb")
            bqr = bqr_pool.tile([128, 1536], dt.bfloat16, tag="bqr",
                                name=f"bqr{p}")
            ta = rope_pool.tile([128, 512], dt.bfloat16, tag="ta")
            tb = rope_pool.tile([128, 512], dt.bfloat16, tag="tb")
            nc.scalar.copy(tmpb[:], ps[:, 0:1024])
            cos4 = (
                cos_sb[:, p * 64 : (p + 1) * 64]
                .unsqueeze(1)
                .broadcast_to([128, 4, 64])
            )
            sin4 = (
                sin_sb[:, p * 64 : (p + 1) * 64]
                .unsqueeze(1)
                .broadcast_to([128, 4, 64])
            )
            cos8 = (
                cos_sb[:, p * 64 : (p + 1) * 64]
                .unsqueeze(1)
                .broadcast_to([128, 8, 64])
            )
            sin8 = (
                sin_sb[:, p * 64 : (p + 1) * 64]
                .unsqueeze(1)
                .broadcast_to([128, 8, 64])
            )
            rope_block(bqr[:, 0:512], stages[p][:, 0:512], cos4, sin4, 4,
                       ta[:, 0:256], tb[:, 0:256])
            rope_block(bqr[:, 512:1536], tmpb[:], cos8, sin8, 8, ta[:], tb[:])
            # bounce roped B_q to the super-tile scratch
            scr = scr_d[p // 2]
            ph = p % 2
            nc.sync.dma_start(
                out=scr[ph * 128 : (ph + 1) * 128, 0:1536], in_=bqr[:]
            )
            bqrs[p] = bqr

        def lv_reads(j):
            """A' diagonal scatter-reads for super-tile j (8 DMAs, gpsimd)."""
            lhs = lhs_bufs[j % 2]
            scr = scr_d[j]
            sa_v = scr[:, 1536:1728].rearrange(
                "(g t) (r h) -> t r g h", t=8, r=RQ
            )
            l_v = lhs[0:96, :].rearrange("(t r) (g c) -> t r g c", t=8, g=32)
            for t in range(8):
                nc.gpsimd.dma_start(
                    out=l_v[t][:, :, t * 16 : (t + 1) * 16], in_=sa_v[t]
                )

        def dv_reads(p):
            """roped-B_q scatter-reads for proj tile p (8 DMAs)."""
            j = p // 2
            ph = p % 2
            bdr = bdr_bufs[j % 2]
            scr = scr_d[j]
            sb_v = scr[ph * 128 : (ph + 1) * 128, 0:1536].rearrange(
                "(g t) (r d) -> t r g d", t=8, r=RQ
            )
            d_v = bdr[0:96, :].rearrange("(t r) (g d) -> t r g d", t=8, g=32)
            for t in range(8):
                eng = nc.sync if t % 4 != 3 else nc.scalar
                eng.dma_start(
                    out=d_v[t][:, ph * 16 : (ph + 1) * 16, :], in_=sb_v[t]
                )

        def consume(p):
            """block-diagonal q contraction + output for proj tile p."""
            j = p // 2
            ph = p % 2
            lhs = lhs_bufs[j % 2]
            bdr = bdr_bufs[j % 2]
            qsb = out_pool.tile([128, 2048], dt.bfloat16, tag="qsb")
            for gq in range(4):
                qp = psq_pool.tile(
                    [128, 512], dt.float32, tag="qp", name=f"qp{p}_{gq}"
                )
                for j4 in range(4):
                    g = ph * 16 + gq * 4 + j4
                    nc.tensor.matmul(
                        qp[:, j4 * 128 : (j4 + 1) * 128],
                        lhs[0:96, g * 128 : (g + 1) * 128],
                        bdr[0:96, g * 128 : (g + 1) * 128],
                        start=True,
                        stop=True,
                    )
                nc.scalar.copy(qsb[:, gq * 512 : (gq + 1) * 512], qp[:])
            nc.scalar.dma_start(out=q_d[p], in_=qsb[:])

        # ================= schedule =================
        # phase 1, stage 1: proj tiles 0,1 k-major, paced by the chunked DMAs
        ps01 = [
            ps_pool.tile([128, 1024], dt.float32, tag="ps", name=f"psA{p}")
            for p in range(2)
        ]
        for kk in range(KT):
            for p in range(2):
                half_mms(p, ps01[p], wa_sb, 0, NA, kk, kk + 1)
        post1(0, ps01[0])
        post1(1, ps01[1])
        # half-B weights can stream now without competing with stage-1 pacing
        for kk in range(KT):
            nc.scalar.dma_start(
                out=wb_sb[:, kk * NB : (kk + 1) * NB], in_=wb_d[kk]
            )
        # phase 1, stage 2: proj tiles 2-7 tile-major (weights resident)
        for p in range(2, NT):
            ps = ps_pool.tile([128, 1024], dt.float32, tag="ps", name=f"psA{p}")
            half_mms(p, ps, wa_sb, 0, NA, 0, KT)
            post1(p, ps)

        # phase 2: half-B + woven BD contraction
        for p in range(NT):
            if p >= 1:
                dv_reads(p - 1)
            ps = ps_pool.tile([128, 1024], dt.float32, tag="ps", name=f"psB{p}")
            half_mms(p, ps, wb_sb, 0, NB, 0, KT)
            post2(p, ps)
            if p % 2 == 0:
                lv_reads(p // 2)
            if p >= 1:
                consume(p - 1)
        dv_reads(NT - 1)
        consume(NT - 1)


def build_program():
    import concourse.tile as tile

    nc, tensors = make_nc()
    with tile.TileContext(nc) as tc:
        build_body(nc, tc, tensors)
    nc.compile()
    return nc


def _get_program():
    if "nc" not in _CACHE:
        _CACHE["nc"] = build_program()
    return _CACHE["nc"]


def make_in_maps(x, W_A_q, W_B_q, W_A_k, W_B_k, W_A_v, W_B_v):
    """Shard + preprocess full inputs into per-core input maps."""
    x = np.asarray(x)
    B, S, Hh = x.shape
    x2 = np.ascontiguousarray(x.reshape(B * S, Hh))

    # fold the 1/RQ scale and the (h,r)->(r,h) column reorder into W_A_q
    WAq = np.asarray(W_A_q).reshape(Hh, NH, RQ).transpose(0, 2, 1).reshape(
        Hh, NH * RQ
    ) / np.float32(RQ)
    WBq = np.asarray(W_B_q)
    Wa = np.concatenate(
        [
            WBq[:, 0:512],
            WAq,
            np.asarray(W_A_k),
            np.asarray(W_A_v),
            np.asarray(W_B_k),
            np.asarray(W_B_v),
        ],
        axis=1,
    )
    Wb = WBq[:, 512:1536]
    assert Wa.shape == (Hh, NA) and Wb.shape == (Hh, NB)
    Wat = np.ascontiguousarray(Wa.reshape(KT, 128, NA)).astype(BF16)
    Wbt = np.ascontiguousarray(Wb.reshape(KT, 128, NB)).astype(BF16)

    inv = 1.0 / (10000.0 ** (np.arange(0, HD, 2, dtype=np.float32) / HD))
    ang = np.arange(S, dtype=np.float32)[:, None] * inv[None, :]
    cos_rep = np.ascontiguousarray(np.cos(ang)).astype(BF16)
    sin_rep = np.ascontiguousarray(np.sin(ang)).astype(BF16)

    in_maps = []
    for i in range(8):
        tok0 = i * SH
        pos = np.arange(tok0, tok0 + SH) % S
        in_maps.append(
            {
                # pre-transposed (hidden, tokens) so on-chip loads are plain
                "x": np.ascontiguousarray(x2[tok0 : tok0 + SH].T).astype(BF16),
                "wa": Wat,
                "wb": Wbt,
                "cosr": np.ascontiguousarray(cos_rep[pos]),
                "sinr": np.ascontiguousarray(sin_rep[pos]),
            }
        )
    return in_maps, (B, S)


def assemble_outputs(results, B, S):
    # q arrives in raw block-diagonal layout: [p, t*16+h, g*128+d] with
    # token = p*128 + g*8 + t
    qs = []
    for i in range(8):
        a = results[i]["q"].astype(np.float32).reshape(NT, 8, 16, 16, 128)
        qs.append(a.transpose(0, 3, 1, 2, 4).reshape(SH, NH, HD))
    q = np.concatenate(qs, axis=0).reshape(B, S, NH, HD)
    k = np.concatenate(
        [results[i]["k"].astype(np.float32) for i in range(8)], axis=0
    ).reshape(B, S, NH, HD)
    v = np.concatenate(
        [results[i]["v"].astype(np.float32) for i in range(8)], axis=0
    ).reshape(B, S, NH, HD)
    return q, k, v


def kernel(x, W_A_q, W_B_q, W_A_k, W_B_k, W_A_v, W_B_v):
    from concourse.bass_utils import run_bass_kernel_spmd

    nc = _get_program()
    in_maps, (B, S) = make_in_maps(x, W_A_q, W_B_q, W_A_k, W_B_k, W_A_v, W_B_v)
    res = run_bass_kernel_spmd(nc, in_maps, list(range(8))).results
    return assemble_outputs(res, B, S)


# revision 7
# speedup vs baseline: 1.1017x; 1.1017x over previous
"""Trainium2 Bass kernel for nn_CPLinear (CP-decomposed QKV projection with RoPE).

Computes, for x:(2,4096,2048) and CP-factor weights:
    A_t = x @ W_A_t  (per-token head coefficients),  B_t = x @ W_B_t (shared bases)
    q = einsum('bshr,bsrd->bshd', A_q, rope(B_q)) / 12
    k = A_k * rope(B_k)   (rank-1)
    v = A_v * B_v         (rank-1)

Strategy (8 cores, data-parallel over the 8192 tokens, 1024 tokens/core):
  - All 6 projections fused into one [2048 x 2016] bf16 matmul; each k-chunk
    runs 4 matmuls off one stationary load (LDWEIGHTS fully amortized).
  - W/x stream in k-chunks split across both HWDGE queues; the first two token
    tiles run k-major, paced by chunk arrival, so the PE starts ~10us in.
  - PSUM: psa[512]x1 + psb[1536]x2 + psq[512]x1 = 8 banks; evictions ordered
    so no projection matmul ever waits on a PSUM release.
  - The rank-12 q contraction runs as block-diagonal matmuls (8 tokens per
    matmul, K=96); its 4 PSUM-bank groups are woven between later tiles'
    k-chunks so the scatter round-trip and qsb evictions hide under PE work.
  - q is written in raw block-diagonal layout and untangled on the host.
"""

import sys

for _p in ("/opt/trn_rl_repo",):
    if _p not in sys.path:
        sys.path.insert(0, _p)

import numpy as np
import ml_dtypes

BF16 = ml_dtypes.bfloat16

SH = 1024          # tokens per core
H = 2048           # hidden
KT = H // 128      # 16 k-tiles
NT = SH // 128     # 8 token tiles per core
NOUT = 2016        # fused projection output width
WSPL = 1504        # k-chunk DMA column split between the two HWDGE queues
NH, HD, RQ = 16, 128, 12

_CACHE = {}


def make_nc():
    import concourse.bacc as bacc
    from concourse import mybir

    dt = mybir.dt

    nc = bacc.Bacc(
        "TRN2",
        target_bir_lowering=False,
        debug=False,
        enable_asserts=False,
        num_devices=8,
    )

    x_d = nc.dram_tensor("x", (H, SH), dt.bfloat16, kind="ExternalInput")  # pre-transposed host-side
    w_d = nc.dram_tensor("w", (KT, 128, NOUT), dt.bfloat16, kind="ExternalInput")
    cos_d = nc.dram_tensor("cosr", (SH, 64), dt.bfloat16, kind="ExternalInput")
    sin_d = nc.dram_tensor("sinr", (SH, 64), dt.bfloat16, kind="ExternalInput")
    q_d = nc.dram_tensor("q", (NT, 128, NH * HD), dt.bfloat16, kind="ExternalOutput")
    k_d = nc.dram_tensor("k", (SH, NH * HD), dt.bfloat16, kind="ExternalOutput")
    v_d = nc.dram_tensor("v", (SH, NH * HD), dt.bfloat16, kind="ExternalOutput")
    return nc, (x_d, w_d, cos_d, sin_d, q_d, k_d, v_d)


def build_body(nc, tc, tensors):
    from contextlib import ExitStack

    from concourse import mybir

    dt = mybir.dt
    x_d, w_d, cos_d, sin_d, q_d, k_d, v_d = tensors

    with ExitStack() as ctx:
        P = ctx.enter_context
        const_pool = P(tc.tile_pool(name="const", bufs=1))
        w_sb = const_pool.tile([128, KT * NOUT], dt.bfloat16, tag="w_sb")
        xT = const_pool.tile([128, KT * SH], dt.bfloat16, tag="xT")
        cos_sb = const_pool.tile([128, NT * 64], dt.bfloat16, tag="cos_sb")
        sin_sb = const_pool.tile([128, NT * 64], dt.bfloat16, tag="sin_sb")
        lhs_bufs = [
            const_pool.tile([128, 2048], dt.bfloat16, tag=f"lhs{i}", name=f"lhs{i}")
            for i in range(3)
        ]
        bdr_bufs = [
            const_pool.tile([128, 2048], dt.bfloat16, tag=f"bdr{i}", name=f"bdr{i}")
            for i in range(3)
        ]

        # ---- startup DMAs: k-chunked, split across both HWDGE queues ----
        nc.gpsimd.dma_start(
            out=cos_sb[:].rearrange("p (t n) -> p t n", t=NT),
            in_=cos_d[:].rearrange("(t p) n -> p t n", p=128),
        )
        nc.gpsimd.dma_start(
            out=sin_sb[:].rearrange("p (t n) -> p t n", t=NT),
            in_=sin_d[:].rearrange("(t p) n -> p t n", p=128),
        )
        for tl in lhs_bufs:
            nc.gpsimd.memset(tl[:], 0.0)
        for kk in range(KT):
            nc.scalar.dma_start(
                out=w_sb[:, kk * NOUT : kk * NOUT + WSPL],
                in_=w_d[kk][:, 0:WSPL],
            )
            nc.sync.dma_start(
                out=xT[:, kk * SH : (kk + 1) * SH],
                in_=x_d[kk * 128 : (kk + 1) * 128, :],
            )
            nc.sync.dma_start(
                out=w_sb[:, kk * NOUT + WSPL : (kk + 1) * NOUT],
                in_=w_d[kk][:, WSPL:NOUT],
            )

        psa_pool = P(tc.tile_pool(name="psa", bufs=1, space="PSUM"))
        psb_pool = P(tc.tile_pool(name="psb", bufs=2, space="PSUM"))
        psq_pool = P(tc.tile_pool(name="psq", bufs=1, space="PSUM"))
        small_pool = P(tc.tile_pool(name="small", bufs=3))
        bq_pool = P(tc.tile_pool(name="bq", bufs=2))
        bqr_pool = P(tc.tile_pool(name="bqr", bufs=2))
        rope_pool = P(tc.tile_pool(name="rope", bufs=3))
        out_pool = P(tc.tile_pool(name="outs", bufs=2))
        scr_pool = P(tc.tile_pool(name="scr", bufs=3, space="DRAM"))

        def proj_chunk(p, ps_a, ps_b, kk):
            t0 = p * 128
            lh = xT[:, kk * SH + t0 : kk * SH + t0 + 128]
            wb = kk * NOUT
            st = kk == 0
            sp = kk == KT - 1
            nc.tensor.matmul(
                ps_a[:, 0:480], lh, w_sb[:, wb : wb + 480], start=st, stop=sp
            )
            for c in range(3):
                nc.tensor.matmul(
                    ps_b[:, c * 512 : (c + 1) * 512],
                    lh,
                    w_sb[:, wb + 480 + c * 512 : wb + 480 + (c + 1) * 512],
                    start=st,
                    stop=sp,
                )

        state = {}

        def post_a(p, ps_a):
            """psa eviction + A' bounce + ropeK + k/v for proj tile p."""
            t0 = p * 128
            scr = scr_pool.tile([128, 1728], dt.bfloat16, tag="scr",
                                name=f"scr{p}")
            smalls = small_pool.tile([128, 480], dt.bfloat16, tag="smalls")
            bkr = small_pool.tile([128, 128], dt.bfloat16, tag="bkr")
            tka = small_pool.tile([128, 64], dt.bfloat16, tag="tka")
            tkb = small_pool.tile([128, 64], dt.bfloat16, tag="tkb")
            nc.scalar.copy(smalls[:], ps_a[:, 0:480])
            # A' -> scratch (read back by l_v)
            nc.gpsimd.dma_start(out=scr[:, 1536:1728], in_=smalls[:, 0:192])

            # rope on B_k (rank-1)
            cos_k = cos_sb[:, p * 64 : (p + 1) * 64]
            sin_k = sin_sb[:, p * 64 : (p + 1) * 64]
            bkv = smalls[:, 224:352].rearrange("p (two d) -> p two d", two=2)
            bkrv = bkr[:].rearrange("p (two d) -> p two d", two=2)
            nc.vector.tensor_mul(tka[:], bkv[:, 0], cos_k)
            nc.vector.tensor_mul(tkb[:], bkv[:, 1], sin_k)
            nc.vector.tensor_sub(bkrv[:, 0], tka[:], tkb[:])
            nc.vector.tensor_mul(tka[:], bkv[:, 1], cos_k)
            nc.vector.tensor_mul(tkb[:], bkv[:, 0], sin_k)
            nc.vector.tensor_add(bkrv[:, 1], tka[:], tkb[:])

            # k, v rank-1 broadcasts + outputs
            ksb = out_pool.tile([128, 2048], dt.bfloat16, tag="ksb")
            vsb = out_pool.tile([128, 2048], dt.bfloat16, tag="vsb")
            nc.vector.tensor_mul(
                ksb[:].rearrange("p (h d) -> p h d", h=NH),
                bkr[:].unsqueeze(1).broadcast_to([128, NH, 128]),
                smalls[:, 192:208].unsqueeze(2).broadcast_to([128, NH, 128]),
            )
            nc.vector.tensor_mul(
                vsb[:].rearrange("p (h d) -> p h d", h=NH),
                smalls[:, 352:480].unsqueeze(1).broadcast_to([128, NH, 128]),
                smalls[:, 208:224].unsqueeze(2).broadcast_to([128, NH, 128]),
            )
            nc.sync.dma_start(out=k_d[t0 : t0 + 128, :], in_=ksb[:])
            nc.sync.dma_start(out=v_d[t0 : t0 + 128, :], in_=vsb[:])
            state[p] = {"scr": scr}

        def post_b(p, ps_b, fast_tail=False):
            """psb eviction, rope on B_q, bounce + scatter reads for tile p."""
            st = state[p]
            scr = st["scr"]
            bq = bq_pool.tile([128, 1536], dt.bfloat16, tag="bq")
            bqr = bqr_pool.tile([128, 1536], dt.bfloat16, tag="bqr")
            ta = rope_pool.tile([128, 768], dt.bfloat16, tag="ta")
            tb = rope_pool.tile([128, 768], dt.bfloat16, tag="tb")
            nc.scalar.copy(bq[:], ps_b[:])
            cosr = (
                cos_sb[:, p * 64 : (p + 1) * 64]
                .unsqueeze(1)
                .broadcast_to([128, RQ, 64])
            )
            sinr = (
                sin_sb[:, p * 64 : (p + 1) * 64]
                .unsqueeze(1)
                .broadcast_to([128, RQ, 64])
            )
            sv = bq[:].rearrange("p (r two d) -> p r two d", r=RQ, two=2)
            dv = bqr[:].rearrange("p (r two d) -> p r two d", r=RQ, two=2)
            tav = ta[:].rearrange("p (r d) -> p r d", r=RQ)
            tbv = tb[:].rearrange("p (r d) -> p r d", r=RQ)
            p_lo = sv[:, :, 0]
            p_hi = sv[:, :, 1]
            nc.vector.tensor_mul(tav, p_lo, cosr)
            nc.vector.tensor_mul(tbv, p_hi, sinr)
            nc.vector.tensor_sub(dv[:, :, 0], tav, tbv)
            nc.vector.tensor_mul(tav, p_hi, cosr)
            nc.vector.tensor_mul(tbv, p_lo, sinr)
            nc.vector.tensor_add(dv[:, :, 1], tav, tbv)

            # bounce roped B_q, then scatter-read the block-diagonal operands
            nc.scalar.dma_start(out=scr[:, 0:1536], in_=bqr[:])
            lhs = lhs_bufs[p % 3]
            bdr = bdr_bufs[p % 3]
            sa_v = scr[:, 1536:1728].rearrange(
                "(g t) (r h) -> t r g h", t=8, r=RQ
            )
            sb_v = scr[:, 0:1536].rearrange("(g t) (r d) -> t r g d", t=8, r=RQ)
            l_v = lhs[0:96, :].rearrange("(t r) (g c) -> t r g c", t=8, g=16)
            d_v = bdr[0:96, :].rearrange("(t r) (g d) -> t r g d", t=8, g=16)
            for t in range(8):
                if fast_tail:
                    eng_l = nc.sync if t % 2 == 0 else nc.scalar
                else:
                    eng_l = nc.gpsimd
                eng_l.dma_start(
                    out=l_v[t][:, :, t * 16 : (t + 1) * 16], in_=sa_v[t]
                )
            for t in range(8):
                if fast_tail:
                    eng = (nc.sync, nc.scalar, nc.gpsimd)[t % 3]
                else:
                    eng = nc.sync if t % 4 != 3 else nc.scalar
                eng.dma_start(out=d_v[t], in_=sb_v[t])
            st["lhs"] = lhs
            st["bdr"] = bdr

        def bd_group(p, gq):
            """one PSUM-bank group (4 block-diagonal matmuls) of tile p's q."""
            st = state[p]
            if gq == 0:
                st["qsb"] = out_pool.tile([128, 2048], dt.bfloat16, tag="qsb",
                                          name=f"qsb{p}")
            lhs, bdr, qsb = st["lhs"], st["bdr"], st["qsb"]
            qp = psq_pool.tile([128, 512], dt.float32, tag="qp",
                               name=f"qp{p}_{gq}")
            for j4 in range(4):
                g = gq * 4 + j4
                nc.tensor.matmul(
                    qp[:, j4 * 128 : (j4 + 1) * 128],
                    lhs[0:96, g * 128 : (g + 1) * 128],
                    bdr[0:96, g * 128 : (g + 1) * 128],
                    start=True,
                    stop=True,
                )
            nc.scalar.copy(qsb[:, gq * 512 : (gq + 1) * 512], qp[:])
            if gq == 3:
                nc.scalar.dma_start(out=q_d[p], in_=qsb[:])

        # ================= schedule =================
        # stage 1: proj tiles 0,1 k-major, paced by the chunked W/x DMAs.
        # tile 1's A-block accumulates in the (otherwise idle) psq pool.
        ps_a0 = psa_pool.tile([128, 512], dt.float32, tag="psa", name="psa0")
        ps_b0 = psb_pool.tile([128, 1536], dt.float32, tag="psb", name="psb0")
        ps_a1 = psq_pool.tile([128, 512], dt.float32, tag="qp", name="psa1")
        ps_b1 = psb_pool.tile([128, 1536], dt.float32, tag="psb", name="psb1")
        for kk in range(KT):
            proj_chunk(0, ps_a0, ps_b0, kk)
            proj_chunk(1, ps_a1, ps_b1, kk)
        post_a(0, ps_a0)
        post_b(0, ps_b0)
        post_a(1, ps_a1)
        post_b(1, ps_b1)

        # stage 2: tiles 2-7 tile-major with the BD contraction woven in.
        # proj(3) carries the BD(0)/BD(1) catch-up; from then on BD(p-1)
        # starts at post(p) and finishes inside proj(p+1).
        weave = {
            3: [(0, 0, 1), (0, 1, 3), (0, 2, 5), (0, 3, 7),
                (1, 0, 9), (1, 1, 11), (1, 2, 13), (1, 3, 15)],
        }
        for p in range(4, NT):
            weave[p] = [(p - 2, 1, 3), (p - 2, 2, 7), (p - 2, 3, 11)]
        for p in range(2, NT):
            ps_a = psa_pool.tile([128, 512], dt.float32, tag="psa",
                                 name=f"psa{p}")
            ps_b = psb_pool.tile([128, 1536], dt.float32, tag="psb",
                                 name=f"psb{p}")
            slots = {kk: (bp, gq) for (bp, gq, kk) in weave.get(p, [])}
            for kk in range(KT):
                proj_chunk(p, ps_a, ps_b, kk)
                if kk in slots:
                    bd_group(*slots[kk])
            post_a(p, ps_a)
            if p >= 3:
                bd_group(p - 1, 0)
            post_b(p, ps_b, fast_tail=(p == NT - 1))
        # tail: BD(6) g1-3 + BD(7) g0-3
        for gq in range(1, 4):
            bd_group(NT - 2, gq)
        for gq in range(4):
            bd_group(NT - 1, gq)


def build_program():
    import concourse.tile as tile

    nc, tensors = make_nc()
    with tile.TileContext(nc) as tc:
        build_body(nc, tc, tensors)
    nc.compile()
    return nc


def _get_program():
    if "nc" not in _CACHE:
        _CACHE["nc"] = build_program()
    return _CACHE["nc"]


def make_in_maps(x, W_A_q, W_B_q, W_A_k, W_B_k, W_A_v, W_B_v):
    """Shard + preprocess full inputs into per-core input maps."""
    x = np.asarray(x)
    B, S, Hh = x.shape
    x2 = np.ascontiguousarray(x.reshape(B * S, Hh))

    # fold the 1/RQ scale and the (h,r)->(r,h) column reorder into W_A_q
    WAq = np.asarray(W_A_q).reshape(Hh, NH, RQ).transpose(0, 2, 1).reshape(
        Hh, NH * RQ
    ) / np.float32(RQ)
    Wall = np.concatenate(
        [
            WAq,
            np.asarray(W_A_k),
            np.asarray(W_A_v),
            np.asarray(W_B_k),
            np.asarray(W_B_v),
            np.asarray(W_B_q),
        ],
        axis=1,
    )
    assert Wall.shape == (Hh, NOUT)
    Wt = np.ascontiguousarray(Wall.reshape(KT, 128, NOUT)).astype(BF16)

    inv = 1.0 / (10000.0 ** (np.arange(0, HD, 2, dtype=np.float32) / HD))
    ang = np.arange(S, dtype=np.float32)[:, None] * inv[None, :]
    cos_rep = np.ascontiguousarray(np.cos(ang)).astype(BF16)
    sin_rep = np.ascontiguousarray(np.sin(ang)).astype(BF16)

    in_maps = []
    for i in range(8):
        tok0 = i * SH
        pos = np.arange(tok0, tok0 + SH) % S
        in_maps.append(
            {
                # pre-transposed (hidden, tokens) so on-chip loads are plain
                "x": np.ascontiguousarray(x2[tok0 : tok0 + SH].T).astype(BF16),
                "w": Wt,
                "cosr": np.ascontiguousarray(cos_rep[pos]),
                "sinr": np.ascontiguousarray(sin_rep[pos]),
            }
        )
    return in_maps, (B, S)


def assemble_outputs(results, B, S):
    # q arrives in raw block-diagonal layout: [p, t*16+h, g*128+d] with
    # token = p*128 + g*8 + t
    qs = []
    for i in range(8):
        a = results[i]["q"].astype(np.float32).reshape(NT, 8, 16, 16, 128)
        qs.append(a.transpose(0, 3, 1, 2, 4).reshape(SH, NH, HD))
    q = np.concatenate(qs, axis=0).reshape(B, S, NH, HD)
    k = np.concatenate(
        [results[i]["k"].astype(np.float32) for i in range(8)], axis=0
    ).reshape(B, S, NH, HD)
    v = np.concatenate(
        [results[i]["v"].astype(np.float32) for i in range(8)], axis=0
    ).reshape(B, S, NH, HD)
    return q, k, v


def kernel(x, W_A_q, W_B_q, W_A_k, W_B_k, W_A_v, W_B_v):
    from concourse.bass_utils import run_bass_kernel_spmd

    nc = _get_program()
    in_maps, (B, S) = make_in_maps(x, W_A_q, W_B_q, W_A_k, W_B_k, W_A_v, W_B_v)
    res = run_bass_kernel_spmd(nc, in_maps, list(range(8))).results
    return assemble_outputs(res, B, S)


# revision 8
# speedup vs baseline: 1.2430x; 1.1282x over previous
"""Trainium2 Bass kernel for nn_CPLinear (CP-decomposed QKV projection with RoPE).

Computes, for x:(2,4096,2048) and CP-factor weights:
    A_t = x @ W_A_t  (per-token head coefficients),  B_t = x @ W_B_t (shared bases)
    q = einsum('bshr,bsrd->bshd', A_q, rope(B_q)) / 12
    k = A_k * rope(B_k)   (rank-1)
    v = A_v * B_v         (rank-1)

Strategy (8 cores, data-parallel over the 8192 tokens, 1024 tokens/core):
  - All 6 projections fused into one [2048 x 2016] bf16 matmul; each k-chunk
    runs 4 matmuls off one stationary load (LDWEIGHTS fully amortized).
  - W/x stream in k-chunks split across both HWDGE queues; the first two token
    tiles run k-major, paced by chunk arrival, so the PE starts ~10us in.
  - PSUM: psa[512]x1 + psb[1536]x2 + psq[512]x1 = 8 banks; evictions ordered
    so no projection matmul ever waits on a PSUM release.
  - The rank-12 q contraction runs as block-diagonal matmuls (8 tokens per
    matmul, K=96); its 4 PSUM-bank groups are woven between later tiles'
    k-chunks so the scatter round-trip and qsb evictions hide under PE work.
  - q is written in raw block-diagonal layout and untangled on the host.
"""

import sys

for _p in ("/opt/trn_rl_repo",):
    if _p not in sys.path:
        sys.path.insert(0, _p)

import numpy as np
import ml_dtypes

BF16 = ml_dtypes.bfloat16

SH = 1024          # tokens per core
H = 2048           # hidden
KT = H // 128      # 16 k-tiles
NT = SH // 128     # 8 token tiles per core
NOUT = 2016        # fused projection output width
WSPL = 1504        # k-chunk DMA column split between the two HWDGE queues
NH, HD, RQ = 16, 128, 12

_CACHE = {}


def make_nc():
    import concourse.bacc as bacc
    from concourse import mybir

    dt = mybir.dt

    nc = bacc.Bacc(
        "TRN2",
        target_bir_lowering=False,
        debug=False,
        enable_asserts=False,
        num_devices=8,
    )

    x_d = nc.dram_tensor("x", (H, SH), dt.bfloat16, kind="ExternalInput")  # pre-transposed host-side
    w_d = nc.dram_tensor("w", (KT, 128, NOUT), dt.bfloat16, kind="ExternalInput")
    cos_d = nc.dram_tensor("cosr", (SH, 64), dt.bfloat16, kind="ExternalInput")
    sin_d = nc.dram_tensor("sinr", (SH, 64), dt.bfloat16, kind="ExternalInput")
    q_d = nc.dram_tensor("q", (NT, 128, NH * HD), dt.bfloat16, kind="ExternalOutput")
    k_d = nc.dram_tensor("k", (SH, NH * HD), dt.bfloat16, kind="ExternalOutput")
    v_d = nc.dram_tensor("v", (SH, NH * HD), dt.bfloat16, kind="ExternalOutput")
    return nc, (x_d, w_d, cos_d, sin_d, q_d, k_d, v_d)


def build_body(nc, tc, tensors):
    from contextlib import ExitStack

    from concourse import mybir

    dt = mybir.dt
    x_d, w_d, cos_d, sin_d, q_d, k_d, v_d = tensors

    with ExitStack() as ctx:
        P = ctx.enter_context
        const_pool = P(tc.tile_pool(name="const", bufs=1))
        w_sb = const_pool.tile([128, KT * NOUT], dt.bfloat16, tag="w_sb")
        xT = const_pool.tile([128, KT * SH], dt.bfloat16, tag="xT")
        cos_sb = const_pool.tile([128, NT * 64], dt.bfloat16, tag="cos_sb")
        sin_sb = const_pool.tile([128, NT * 64], dt.bfloat16, tag="sin_sb")
        lhs_bufs = [
            const_pool.tile([128, 2048], dt.bfloat16, tag=f"lhs{i}", name=f"lhs{i}")
            for i in range(3)
        ]
        bdr_bufs = [
            const_pool.tile([128, 2048], dt.bfloat16, tag=f"bdr{i}", name=f"bdr{i}")
            for i in range(3)
        ]

        # ---- startup DMAs: k-chunked, split across both HWDGE queues ----
        nc.gpsimd.dma_start(
            out=cos_sb[:].rearrange("p (t n) -> p t n", t=NT),
            in_=cos_d[:].rearrange("(t p) n -> p t n", p=128),
        )
        nc.gpsimd.dma_start(
            out=sin_sb[:].rearrange("p (t n) -> p t n", t=NT),
            in_=sin_d[:].rearrange("(t p) n -> p t n", p=128),
        )
        for tl in lhs_bufs:
            nc.gpsimd.memset(tl[:], 0.0)
        for kk in range(KT):
            nc.scalar.dma_start(
                out=w_sb[:, kk * NOUT : kk * NOUT + WSPL],
                in_=w_d[kk][:, 0:WSPL],
            )
            nc.sync.dma_start(
                out=xT[:, kk * SH : (kk + 1) * SH],
                in_=x_d[kk * 128 : (kk + 1) * 128, :],
            )
            nc.sync.dma_start(
                out=w_sb[:, kk * NOUT + WSPL : (kk + 1) * NOUT],
                in_=w_d[kk][:, WSPL:NOUT],
            )

        psa_pool = P(tc.tile_pool(name="psa", bufs=1, space="PSUM"))
        psb_pool = P(tc.tile_pool(name="psb", bufs=2, space="PSUM"))
        psq_pool = P(tc.tile_pool(name="psq", bufs=1, space="PSUM"))
        small_pool = P(tc.tile_pool(name="small", bufs=3))
        bq_pool = P(tc.tile_pool(name="bq", bufs=2))
        bqr_pool = P(tc.tile_pool(name="bqr", bufs=2))
        rope_pool = P(tc.tile_pool(name="rope", bufs=3))
        out_pool = P(tc.tile_pool(name="outs", bufs=2))
        scr_pool = P(tc.tile_pool(name="scr", bufs=3, space="DRAM"))

        def proj_chunk(p, ps_a, ps_b, kk):
            t0 = p * 128
            lh = xT[:, kk * SH + t0 : kk * SH + t0 + 128]
            wb = kk * NOUT
            st = kk == 0
            sp = kk == KT - 1
            nc.tensor.matmul(
                ps_a[:, 0:480], lh, w_sb[:, wb : wb + 480], start=st, stop=sp
            )
            for c in range(3):
                nc.tensor.matmul(
                    ps_b[:, c * 512 : (c + 1) * 512],
                    lh,
                    w_sb[:, wb + 480 + c * 512 : wb + 480 + (c + 1) * 512],
                    start=st,
                    stop=sp,
                )

        state = {}

        def post_a(p, ps_a):
            """psa eviction + A' bounce + ropeK + k/v for proj tile p."""
            t0 = p * 128
            scr = scr_pool.tile([128, 1728], dt.bfloat16, tag="scr",
                                name=f"scr{p}")
            smalls = small_pool.tile([128, 480], dt.bfloat16, tag="smalls")
            bkr = small_pool.tile([128, 128], dt.bfloat16, tag="bkr")
            tka = small_pool.tile([128, 64], dt.bfloat16, tag="tka")
            tkb = small_pool.tile([128, 64], dt.bfloat16, tag="tkb")
            nc.scalar.copy(smalls[:], ps_a[:, 0:480])
            # A' -> scratch (read back by l_v)
            nc.gpsimd.dma_start(out=scr[:, 1536:1728], in_=smalls[:, 0:192])
            state[p] = {"scr": scr, "smalls": smalls, "bkr": bkr,
                        "tka": tka, "tkb": tkb}

        def post_kv(p):
            """ropeK + k/v rank-1 broadcasts + outputs for proj tile p."""
            t0 = p * 128
            st = state[p]
            smalls, bkr, tka, tkb = (st["smalls"], st["bkr"], st["tka"],
                                     st["tkb"])
            cos_k = cos_sb[:, p * 64 : (p + 1) * 64]
            sin_k = sin_sb[:, p * 64 : (p + 1) * 64]
            bkv = smalls[:, 224:352].rearrange("p (two d) -> p two d", two=2)
            bkrv = bkr[:].rearrange("p (two d) -> p two d", two=2)
            nc.vector.tensor_mul(tka[:], bkv[:, 0], cos_k)
            nc.vector.tensor_mul(tkb[:], bkv[:, 1], sin_k)
            nc.vector.tensor_sub(bkrv[:, 0], tka[:], tkb[:])
            nc.vector.tensor_mul(tka[:], bkv[:, 1], cos_k)
            nc.vector.tensor_mul(tkb[:], bkv[:, 0], sin_k)
            nc.vector.tensor_add(bkrv[:, 1], tka[:], tkb[:])
            ksb = out_pool.tile([128, 2048], dt.bfloat16, tag="ksb")
            vsb = out_pool.tile([128, 2048], dt.bfloat16, tag="vsb")
            nc.vector.tensor_mul(
                ksb[:].rearrange("p (h d) -> p h d", h=NH),
                bkr[:].unsqueeze(1).broadcast_to([128, NH, 128]),
                smalls[:, 192:208].unsqueeze(2).broadcast_to([128, NH, 128]),
            )
            nc.vector.tensor_mul(
                vsb[:].rearrange("p (h d) -> p h d", h=NH),
                smalls[:, 352:480].unsqueeze(1).broadcast_to([128, NH, 128]),
                smalls[:, 208:224].unsqueeze(2).broadcast_to([128, NH, 128]),
            )
            nc.sync.dma_start(out=k_d[t0 : t0 + 128, :], in_=ksb[:])
            nc.sync.dma_start(out=v_d[t0 : t0 + 128, :], in_=vsb[:])

        def post_b(p, ps_b, fast_tail=False):
            """psb eviction, rope on B_q, bounce + scatter reads for tile p."""
            st = state[p]
            scr = st["scr"]
            bq = bq_pool.tile([128, 1536], dt.bfloat16, tag="bq")
            bqr = bqr_pool.tile([128, 1536], dt.bfloat16, tag="bqr")
            ta = rope_pool.tile([128, 768], dt.bfloat16, tag="ta")
            tb = rope_pool.tile([128, 768], dt.bfloat16, tag="tb")
            nc.scalar.copy(bq[:], ps_b[:])
            cosr = (
                cos_sb[:, p * 64 : (p + 1) * 64]
                .unsqueeze(1)
                .broadcast_to([128, RQ, 64])
            )
            sinr = (
                sin_sb[:, p * 64 : (p + 1) * 64]
                .unsqueeze(1)
                .broadcast_to([128, RQ, 64])
            )
            sv = bq[:].rearrange("p (r two d) -> p r two d", r=RQ, two=2)
            dv = bqr[:].rearrange("p (r two d) -> p r two d", r=RQ, two=2)
            tav = ta[:].rearrange("p (r d) -> p r d", r=RQ)
            tbv = tb[:].rearrange("p (r d) -> p r d", r=RQ)
            p_lo = sv[:, :, 0]
            p_hi = sv[:, :, 1]
            nc.vector.tensor_mul(tav, p_lo, cosr)
            nc.vector.tensor_mul(tbv, p_hi, sinr)
            nc.vector.tensor_sub(dv[:, :, 0], tav, tbv)
            nc.vector.tensor_mul(tav, p_hi, cosr)
            nc.vector.tensor_mul(tbv, p_lo, sinr)
            nc.vector.tensor_add(dv[:, :, 1], tav, tbv)

            # bounce roped B_q, then scatter-read the block-diagonal operands
            nc.scalar.dma_start(out=scr[:, 0:1536], in_=bqr[:])
            lhs = lhs_bufs[p % 3]
            bdr = bdr_bufs[p % 3]
            sa_v = scr[:, 1536:1728].rearrange(
                "(g t) (r h) -> t r g h", t=8, r=RQ
            )
            sb_v = scr[:, 0:1536].rearrange("(g t) (r d) -> t r g d", t=8, r=RQ)
            l_v = lhs[0:96, :].rearrange("(t r) (g c) -> t r g c", t=8, g=16)
            d_v = bdr[0:96, :].rearrange("(t r) (g d) -> t r g d", t=8, g=16)
            for t in range(8):
                if fast_tail:
                    eng_l = nc.sync if t % 2 == 0 else nc.scalar
                else:
                    eng_l = nc.gpsimd
                eng_l.dma_start(
                    out=l_v[t][:, :, t * 16 : (t + 1) * 16], in_=sa_v[t]
                )
            for t in range(8):
                if fast_tail:
                    eng = (nc.sync, nc.scalar, nc.gpsimd)[t % 3]
                else:
                    eng = (nc.sync, nc.scalar, nc.gpsimd)[t % 3]
                eng.dma_start(out=d_v[t], in_=sb_v[t])
            st["lhs"] = lhs
            st["bdr"] = bdr

        def bd_group(p, gq):
            """one PSUM-bank group (4 block-diagonal matmuls) of tile p's q."""
            st = state[p]
            if gq == 0:
                st["qsb"] = out_pool.tile([128, 2048], dt.bfloat16, tag="qsb",
                                          name=f"qsb{p}")
            lhs, bdr, qsb = st["lhs"], st["bdr"], st["qsb"]
            qp = psq_pool.tile([128, 512], dt.float32, tag="qp",
                               name=f"qp{p}_{gq}")
            for j4 in range(4):
                g = gq * 4 + j4
                nc.tensor.matmul(
                    qp[:, j4 * 128 : (j4 + 1) * 128],
                    lhs[0:96, g * 128 : (g + 1) * 128],
                    bdr[0:96, g * 128 : (g + 1) * 128],
                    start=True,
                    stop=True,
                )
            nc.scalar.copy(qsb[:, gq * 512 : (gq + 1) * 512], qp[:])
            if gq == 3:
                nc.scalar.dma_start(out=q_d[p], in_=qsb[:])

        # ================= schedule =================
        # stage 1: proj tiles 0,1 k-major, paced by the chunked W/x DMAs.
        # tile 1's A-block accumulates in the (otherwise idle) psq pool.
        ps_a0 = psa_pool.tile([128, 512], dt.float32, tag="psa", name="psa0")
        ps_b0 = psb_pool.tile([128, 1536], dt.float32, tag="psb", name="psb0")
        ps_a1 = psq_pool.tile([128, 512], dt.float32, tag="qp", name="psa1")
        ps_b1 = psb_pool.tile([128, 1536], dt.float32, tag="psb", name="psb1")
        for kk in range(KT):
            proj_chunk(0, ps_a0, ps_b0, kk)
            proj_chunk(1, ps_a1, ps_b1, kk)
        post_a(0, ps_a0)
        post_b(0, ps_b0)
        post_a(1, ps_a1)
        post_b(1, ps_b1)
        post_kv(0)
        post_kv(1)

        # stage 2: tiles 2-7 tile-major with the BD contraction woven in.
        # proj(3) carries the BD(0)/BD(1) catch-up; from then on BD(p-1)
        # starts at post(p) and finishes inside proj(p+1).
        weave = {
            3: [(0, 0, 1), (0, 1, 3), (0, 2, 5), (0, 3, 7),
                (1, 0, 9), (1, 1, 11), (1, 2, 13), (1, 3, 15)],
        }
        for p in range(4, NT):
            weave[p] = [(p - 2, 1, 3), (p - 2, 2, 7), (p - 2, 3, 11)]
        for p in range(2, NT):
            ps_a = psa_pool.tile([128, 512], dt.float32, tag="psa",
                                 name=f"psa{p}")
            ps_b = psb_pool.tile([128, 1536], dt.float32, tag="psb",
                                 name=f"psb{p}")
            slots = {kk: (bp, gq) for (bp, gq, kk) in weave.get(p, [])}
            for kk in range(KT):
                proj_chunk(p, ps_a, ps_b, kk)
                if kk in slots:
                    bd_group(*slots[kk])
            post_a(p, ps_a)
            post_b(p, ps_b, fast_tail=(p == NT - 1))
            if p >= 3:
                bd_group(p - 1, 0)
            post_kv(p)
        # tail: BD(6) g1-3 + BD(7) g0-3, qevicts pipelined on a free ACT
        for gq in range(1, 4):
            bd_group(NT - 2, gq)
        for gq in range(4):
            bd_group(NT - 1, gq)


def build_program():
    import concourse.tile as tile

    nc, tensors = make_nc()
    with tile.TileContext(nc) as tc:
        build_body(nc, tc, tensors)
    nc.compile()
    return nc


def _get_program():
    if "nc" not in _CACHE:
        _CACHE["nc"] = build_program()
    return _CACHE["nc"]


def make_in_maps(x, W_A_q, W_B_q, W_A_k, W_B_k, W_A_v, W_B_v):
    """Shard + preprocess full inputs into per-core input maps."""
    x = np.asarray(x)
    B, S, Hh = x.shape
    x2 = np.ascontiguousarray(x.reshape(B * S, Hh))

    # fold the 1/RQ scale and the (h,r)->(r,h) column reorder into W_A_q
    WAq = np.asarray(W_A_q).reshape(Hh, NH, RQ).transpose(0, 2, 1).reshape(
        Hh, NH * RQ
    ) / np.float32(RQ)
    Wall = np.concatenate(
        [
            WAq,
            np.asarray(W_A_k),
            np.asarray(W_A_v),
            np.asarray(W_B_k),
            np.asarray(W_B_v),
            np.asarray(W_B_q),
        ],
        axis=1,
    )
    assert Wall.shape == (Hh, NOUT)
    Wt = np.ascontiguousarray(Wall.reshape(KT, 128, NOUT)).astype(BF16)

    inv = 1.0 / (10000.0 ** (np.arange(0, HD, 2, dtype=np.float32) / HD))
    ang = np.arange(S, dtype=np.float32)[:, None] * inv[None, :]
    cos_rep = np.ascontiguousarray(np.cos(ang)).astype(BF16)
    sin_rep = np.ascontiguousarray(np.sin(ang)).astype(BF16)

    in_maps = []
    for i in range(8):
        tok0 = i * SH
        pos = np.arange(tok0, tok0 + SH) % S
        in_maps.append(
            {
                # pre-transposed (hidden, tokens) so on-chip loads are plain
                "x": np.ascontiguousarray(x2[tok0 : tok0 + SH].T).astype(BF16),
                "w": Wt,
                "cosr": np.ascontiguousarray(cos_rep[pos]),
                "sinr": np.ascontiguousarray(sin_rep[pos]),
            }
        )
    return in_maps, (B, S)


def assemble_outputs(results, B, S):
    # q arrives in raw block-diagonal layout: [p, t*16+h, g*128+d] with
    # token = p*128 + g*8 + t
    qs = []
    for i in range(8):
        a = results[i]["q"].astype(np.float32).reshape(NT, 8, 16, 16, 128)
        qs.append(a.transpose(0, 3, 1, 2, 4).reshape(SH, NH, HD))
    q = np.concatenate(qs, axis=0).reshape(B, S, NH, HD)
    k = np.concatenate(
        [results[i]["k"].astype(np.float32) for i in range(8)], axis=0
    ).reshape(B, S, NH, HD)
    v = np.concatenate(
        [results[i]["v"].astype(np.float32) for i in range(8)], axis=0
    ).reshape(B, S, NH, HD)
    return q, k, v


def kernel(x, W_A_q, W_B_q, W_A_k, W_B_k, W_A_v, W_B_v):
    from concourse.bass_utils import run_bass_kernel_spmd

    nc = _get_program()
    in_maps, (B, S) = make_in_maps(x, W_A_q, W_B_q, W_A_k, W_B_k, W_A_v, W_B_v)
    res = run_bass_kernel_spmd(nc, in_maps, list(range(8))).results
    return assemble_outputs(res, B, S)
